# revision 21
# baseline (speedup 1.0000x reference)
"""DualAxisAggAttn Trainium2 kernel (8-core data-parallel over batch).

Reference computation per axis A in {W, H} (x: [B,C,H,W], O = 1+2C):
  qkv = conv1x1(x)                  -> q [B,1,H,W], k,v [B,C,H,W]
  s   = softmax(q, axis=A)
  ctx = sum_A(k * s)                -> [B,C,H,1] or [B,C,1,W]
  out = x + sigmoid(v) * ctx
  y   = SiLU(BN(dwconv3x3(out)))
  x'  = out + y
Axes applied sequentially (W then H).

Kernel strategy per core (2 images):
  - layout: channels on partitions (2 halves of 128), flat h*w on free dim
  - algebraic rewrite: ctx = Wk @ xs + bk with xs[c,a] = sum_A x[c,...]*s[...]
    (k never materialized; q bias dropped: softmax is shift-invariant)
  - PE, fp32r (~1e-4 rounding, 4x faster than fp32): v/q matmuls, ctx matmul,
    9 depthwise taps as diagonal-weight matmuls accumulated in PSUM
  - ACT: tanh((v+bv)/2) (sigmoid via tanh keeps bulk ACT ops in the silu act
    table), SiLU(bn) eviction, softmax exp, psum evictions
  - DVE: xs multiply+reduce, (tanh+1)*ctx2 via scalar_tensor_tensor,
    residual adds via affine_then_add
  - POOL: partition_broadcast of scores, f32->f32r conversion, diag build
  - weights transposed / BN-folded on host (one-time prep)
"""

import numpy as np

B, C, H, W = 16, 256, 80, 80
O = 1 + 2 * C
NCORES = 8
BPC = B // NCORES
HW = H * W
PH, PW = H + 2, W + 2
RCH = 6  # rows per v/dw chunk
NCHUNK = (H + RCH - 1) // RCH  # 14
XSCH = 10  # rows per xs chunk -> 8 chunks of 800
NXS = H // XSCH
QCH = 5  # q rows per chunk -> 16 chunks of 400
BN_EPS = 1e-5

_CACHE = {}


def _build(n_img=BPC):
    import concourse.bass as bass
    import concourse.bacc as bacc
    import concourse.mybir as mybir
    import concourse.tile as tile
    from concourse import library_config
    from concourse.masks import make_identity

    f32 = mybir.dt.float32
    f32r = mybir.dt.float32r
    Alu = mybir.AluOpType
    Act = mybir.ActivationFunctionType

    nc = bacc.Bacc("TRN2", target_bir_lowering=False, debug=False)

    xd = nc.declare_dram_parameter("x", [n_img, C, HW], f32, isOutput=False)
    prm = {}
    for st in ("W", "H"):
        prm[st] = {
            "wvT": nc.declare_dram_parameter(f"wvT_{st}", [C, C], f32, isOutput=False),
            "wkT": nc.declare_dram_parameter(f"wkT_{st}", [C, C], f32, isOutput=False),
            "wqT": nc.declare_dram_parameter(f"wqT_{st}", [C, 1], f32, isOutput=False),
            "bv": nc.declare_dram_parameter(f"bv_{st}", [C], f32, isOutput=False),
            "bk": nc.declare_dram_parameter(f"bk_{st}", [C], f32, isOutput=False),
            "dwc": nc.declare_dram_parameter(f"dwc_{st}", [2, 128, 9], f32, isOutput=False),
            "bns": nc.declare_dram_parameter(f"bns_{st}", [C], f32, isOutput=False),
            "bnsh": nc.declare_dram_parameter(f"bnsh_{st}", [C], f32, isOutput=False),
        }
    outd = nc.declare_dram_parameter("out", [n_img, C, HW], f32, isOutput=True)

    with tile.TileContext(nc) as tc:
        with (
            tc.tile_pool(name="wgt", bufs=1) as wgt,
            tc.tile_pool(name="wstage", bufs=2) as wstage,
            tc.tile_pool(name="xin", bufs=1) as xin_pool,
            tc.tile_pool(name="xstage", bufs=2) as xstage_pool,
            tc.tile_pool(name="pad", bufs=1) as pad_pool,
            tc.tile_pool(name="sb2", bufs=2) as sb2,
            tc.tile_pool(name="chk", bufs=2) as chk,
            tc.tile_pool(name="sml", bufs=2) as sml,
            tc.tile_pool(name="oq", bufs=2) as oq,
            tc.tile_pool(name="ps_q", bufs=2, space="PSUM") as ps_q,
            tc.tile_pool(name="ps_v", bufs=2, space="PSUM") as ps_v,
            tc.tile_pool(name="ps_dw", bufs=2, space="PSUM") as ps_dw,
            tc.tile_pool(name="ps_x", bufs=1, space="PSUM") as ps_x,
        ):
            nc.gpsimd.load_library(library_config.attn)
            lp = lambda: nc.allow_low_precision(reason="f32r rounding ~1e-4 acceptable")

            ident = wgt.tile([128, 128], f32)
            make_identity(nc, ident)
            zeros = wgt.tile([128, PW], f32)
            nc.vector.memset(zeros[:], 0.0)

            # ---- per-stage constants ----
            SW = {}
            for st in ("W", "H"):
                p = prm[st]
                lv, lk, lq = [], [], []
                for kt in range(2):
                    stg = wstage.tile([128, C], f32, tag="wstg")
                    nc.scalar.dma_start(out=stg[:], in_=p["wvT"][kt * 128 : (kt + 1) * 128, :])
                    tv = wgt.tile([128, C], f32r, tag=f"lv{st}{kt}")
                    nc.vector.tensor_copy(out=tv[:], in_=stg[:])
                    lv.append(tv)
                    stg2 = wstage.tile([128, C], f32, tag="wstg")
                    nc.scalar.dma_start(out=stg2[:], in_=p["wkT"][kt * 128 : (kt + 1) * 128, :])
                    tk = wgt.tile([128, C], f32r, tag=f"lk{st}{kt}")
                    nc.vector.tensor_copy(out=tk[:], in_=stg2[:])
                    lk.append(tk)
                    stg3 = wstage.tile([128, 1], f32, tag="wstgq")
                    nc.scalar.dma_start(out=stg3[:], in_=p["wqT"][kt * 128 : (kt + 1) * 128, :])
                    tq = wgt.tile([128, 1], f32r, tag=f"lq{st}{kt}")
                    nc.vector.tensor_copy(out=tq[:], in_=stg3[:])
                    lq.append(tq)
                bv, bk, bns, bnsh = [], [], [], []
                for mt in range(2):
                    sl = slice(mt * 128, (mt + 1) * 128)
                    for name, lst in (("bv", bv), ("bk", bk), ("bns", bns), ("bnsh", bnsh)):
                        t = wgt.tile([128, 1], f32, tag=f"{name}{st}{mt}")
                        nc.scalar.dma_start(out=t[:], in_=p[name][sl][:, None])
                        lst.append(t)
                dwco = []
                for mt in range(2):
                    cot = wgt.tile([128, 9], f32, tag=f"dwco{st}{mt}")
                    nc.scalar.dma_start(out=cot[:], in_=p["dwc"][mt])
                    dwco.append(cot)
                SW[st] = dict(lv=lv, lk=lk, lq=lq, bv=bv, bk=bk, dwco=dwco, bns=bns, bnsh=bnsh)

            # ---- per image ----
            for img in range(n_img):
                xin = []
                for mt in range(2):
                    t = xin_pool.tile([128, HW], f32r, tag=f"xin{mt}", name=f"xin{mt}")
                    for sp in range(4):
                        stg = xstage_pool.tile([128, 1600], f32, tag="xstg", name="xstg")
                        nc.sync.dma_start(
                            out=stg[:],
                            in_=xd[img, mt * 128 : (mt + 1) * 128, sp * 1600 : (sp + 1) * 1600],
                        )
                        nc.gpsimd.tensor_copy(
                            out=t[:, sp * 1600 : (sp + 1) * 1600], in_=stg[:]
                        )
                    xin.append(t)

                cur = xin
                for st in ("W", "H"):
                    sw = SW[st]
                    # ---------- q = Wq.T @ x ----------
                    q_rows = sml.tile([80, 80], f32, tag="qrows")
                    for ch in range(16):
                        n0 = ch * QCH * W
                        pq = ps_q.tile([1, QCH * W], f32, tag="psq")
                        for kt in range(2):
                            nc.tensor.matmul(
                                pq[:],
                                sw["lq"][kt][:],
                                cur[kt][:, n0 : n0 + QCH * W],
                                start=(kt == 0),
                                stop=(kt == 1),
                            )
                        qe = sml.tile([1, QCH * W], f32, tag="qe")
                        nc.scalar.copy(qe[:], pq[:])
                        nc.gpsimd.dma_start(
                            out=q_rows[ch * QCH : (ch + 1) * QCH, :], in_=qe[:]
                        )
                    # ---------- softmax ----------
                    if st == "H":
                        ptr = ps_x.tile([80, 80], f32, tag="ptr")
                        nc.tensor.transpose(ptr[:], q_rows[:], ident[:80, :80])
                        q_sm = sml.tile([80, 80], f32, tag="qsm")
                        nc.scalar.copy(q_sm[:], ptr[:])
                    else:
                        q_sm = q_rows
                    nmax = sml.tile([80, 1], f32, tag="nmax")
                    nc.vector.tensor_reduce(
                        nmax[:], q_sm[:], axis=mybir.AxisListType.X, op=Alu.max, negate=True
                    )
                    eq = sml.tile([80, 80], f32, tag="eq")
                    nc.scalar.activation(eq[:], q_sm[:], Act.Exp, bias=nmax[:], scale=1.0)
                    ssum = sml.tile([80, 1], f32, tag="ssum")
                    nc.vector.reduce_sum(ssum[:], eq[:], axis=mybir.AxisListType.X)
                    rs = sml.tile([80, 1], f32, tag="rs")
                    nc.vector.reciprocal(rs[:], ssum[:])
                    s_sm = sml.tile([80, 80], f32, tag="ssm")
                    nc.vector.tensor_scalar_mul(s_sm[:], in0=eq[:], scalar1=rs[:])
                    if st == "H":
                        ptr2 = ps_x.tile([80, 80], f32, tag="ptr")
                        nc.tensor.transpose(ptr2[:], s_sm[:], ident[:80, :80])
                        s_hw = sml.tile([80, 80], f32, tag="shw")
                        nc.scalar.copy(s_hw[:], ptr2[:])
                    else:
                        s_hw = s_sm
                    # ---------- xs = reduce_A(x * s_bcast) ----------
                    xs = [
                        sml.tile([128, H], f32r, tag=f"xs{kt}", name=f"xs{kt}")
                        for kt in range(2)
                    ]
                    if st == "H":
                        xsp = [
                            [
                                sml.tile([128, W], f32, tag=f"xsp{kt}{i}", name=f"xsp{kt}{i}")
                                for i in range(NXS)
                            ]
                            for kt in range(2)
                        ]
                    for ci in range(NXS):
                        c0 = ci * XSCH * W
                        sch = sml.tile([1, XSCH * W], f32, tag="sch")
                        nc.gpsimd.dma_start(
                            out=sch[:], in_=s_hw[ci * XSCH : (ci + 1) * XSCH, :]
                        )
                        sb = sb2.tile([128, XSCH * W], f32, tag="sbc")
                        nc.gpsimd.partition_broadcast(sb[:], sch[:])
                        for kt in range(2):
                            pr = sb2.tile([128, XSCH * W], f32, tag="prod", bufs=1)
                            nc.vector.tensor_mul(pr[:], cur[kt][:, c0 : c0 + XSCH * W], sb[:])
                            if st == "W":
                                with lp():
                                    nc.vector.reduce_sum(
                                        xs[kt][:, ci * XSCH : (ci + 1) * XSCH],
                                        pr.rearrange("p (h w) -> p h w", h=XSCH),
                                        axis=mybir.AxisListType.X,
                                    )
                            else:
                                nc.vector.reduce_sum(
                                    xsp[kt][ci][:],
                                    pr.rearrange("p (h w) -> p w h", h=XSCH),
                                    axis=mybir.AxisListType.X,
                                )
                    if st == "H":
                        for kt in range(2):
                            for i in range(1, NXS - 1):
                                nc.vector.tensor_add(xsp[kt][0][:], xsp[kt][0][:], xsp[kt][i][:])
                            with lp():
                                nc.vector.tensor_add(xs[kt][:], xsp[kt][0][:], xsp[kt][NXS - 1][:])
                    # ---------- ctx2 = 0.5*(Wk @ xs) + 0.5*bk ----------
                    ctx = []
                    for mt in range(2):
                        pc = ps_x.tile([128, H], f32, tag="psctx")
                        for kt in range(2):
                            nc.tensor.matmul(
                                pc[:],
                                sw["lk"][kt][:, mt * 128 : (mt + 1) * 128],
                                xs[kt][:],
                                start=(kt == 0),
                                stop=(kt == 1),
                            )
                        ct = sml.tile([128, H], f32, tag=f"ctx{mt}")
                        nc.scalar.activation(
                            ct[:], pc[:], Act.Identity, bias=sw["bk"][mt][:], scale=0.5
                        )
                        ctx.append(ct)

                    # ---------- diag dw weights (f32r, built on POOL) ----------
                    diag = []
                    for mt in range(2):
                        dd = []
                        for t9 in range(9):
                            d = sml.tile(
                                [128, 128], f32r, tag=f"diag{t9}", bufs=1, name=f"diag{t9}"
                            )
                            nc.gpsimd.tensor_scalar_mul(
                                d[:], in0=ident[:], scalar1=sw["dwco"][mt][:, t9 : t9 + 1]
                            )
                            dd.append(d)
                        diag.append(dd)

                    # ---------- padded 'out' tiles (borders zeroed) ----------
                    padded = []
                    for mt in range(2):
                        pt = pad_pool.tile([128, PH, PW], f32r, tag=f"pad{mt}", name=f"pad{mt}")
                        nc.gpsimd.tensor_copy(out=pt[:, 0, :], in_=zeros[:])
                        nc.gpsimd.tensor_copy(out=pt[:, PH - 1, :], in_=zeros[:])
                        nc.gpsimd.tensor_copy(
                            out=pt[:, 1 : PH - 1, 0:1], in_=zeros[:, : PH - 2][:, :, None]
                        )
                        nc.gpsimd.tensor_copy(
                            out=pt[:, 1 : PH - 1, PW - 1 : PW], in_=zeros[:, : PH - 2][:, :, None]
                        )
                        padded.append(pt)

                    # ---------- v chunks -> tanh -> t2 -> out ----------
                    for mt in range(2):
                        for ch in range(NCHUNK):
                            r0 = ch * RCH
                            nr = min(RCH, H - r0)
                            n0 = r0 * W
                            nn = nr * W
                            pv = ps_v.tile([128, RCH * W], f32, tag="psv")
                            for kt in range(2):
                                nc.tensor.matmul(
                                    pv[:, :nn],
                                    sw["lv"][kt][:, mt * 128 : (mt + 1) * 128],
                                    cur[kt][:, n0 : n0 + nn],
                                    start=(kt == 0),
                                    stop=(kt == 1),
                                )
                            th = chk.tile([128, RCH * W], f32, tag="th")
                            nc.scalar.activation(
                                th[:, :nn], pv[:, :nn], Act.Tanh, bias=sw["bv"][mt][:], scale=0.5
                            )
                            t2 = chk.tile([128, RCH, W], f32, tag="t2")
                            if st == "W":
                                cb = ctx[mt][:, r0 : r0 + nr].broadcast_to([128, nr, W])
                            else:
                                cb = ctx[mt][:, None, :].broadcast_to([128, nr, W])
                            # t2 = (th + 1) * ctx2  == sigmoid(v)*ctx
                            nc.vector.scalar_tensor_tensor(
                                t2[:, :nr, :],
                                in0=th[:, :nn].rearrange("p (h w) -> p h w", w=W),
                                scalar=1.0,
                                in1=cb,
                                op0=Alu.add,
                                op1=Alu.mult,
                            )
                            nc.vector.affine_then_add(
                                padded[mt][:, 1 + r0 : 1 + r0 + nr, 1 : 1 + W],
                                cur[mt][:, n0 : n0 + nn].rearrange("p (h w) -> p h w", w=W),
                                t2[:, :nr, :],
                                scale=1.0,
                                bias=0.0,
                            )

                    # ---------- depthwise conv (PE) + BN+SiLU + residual ----------
                    for mt in range(2):
                        for ch in range(NCHUNK):
                            r0 = ch * RCH
                            nr = min(RCH, H - r0)
                            n0 = r0 * W
                            nn = nr * W
                            pdw = ps_dw.tile([128, RCH * W], f32, tag="psdw")
                            t9 = 0
                            for dy in (-1, 0, 1):
                                for dx in (-1, 0, 1):
                                    rhs = padded[mt][
                                        :, 1 + r0 + dy : 1 + r0 + dy + nr, 1 + dx : 1 + dx + W
                                    ]
                                    nc.tensor.matmul(
                                        pdw[:, :nn],
                                        diag[mt][t9][:],
                                        rhs,
                                        start=(t9 == 0),
                                        stop=(t9 == 8),
                                    )
                                    t9 += 1
                            ysil = chk.tile([128, RCH * W], f32, tag="ysil")
                            nc.scalar.activation(
                                ysil[:, :nn], pdw[:, :nn], Act.Silu,
                                bias=sw["bnsh"][mt][:], scale=sw["bns"][mt][:],
                            )
                            if st == "W":
                                with lp():
                                    nc.vector.affine_then_add(
                                        cur[mt][:, n0 : n0 + nn].rearrange("p (h w) -> p h w", w=W),
                                        padded[mt][:, 1 + r0 : 1 + r0 + nr, 1 : 1 + W],
                                        ysil[:, :nn].rearrange("p (h w) -> p h w", w=W),
                                        scale=1.0,
                                        bias=0.0,
                                    )
                            else:
                                och = oq.tile([128, RCH * W], f32, tag="och")
                                nc.vector.affine_then_add(
                                    och[:, :nn].rearrange("p (h w) -> p h w", w=W),
                                    padded[mt][:, 1 + r0 : 1 + r0 + nr, 1 : 1 + W],
                                    ysil[:, :nn].rearrange("p (h w) -> p h w", w=W),
                                    scale=1.0,
                                    bias=0.0,
                                )
                                nc.gpsimd.dma_start(
                                    out=outd[img, mt * 128 : (mt + 1) * 128, n0 : n0 + nn],
                                    in_=och[:, :nn],
                                )

    nc.finalize()
    return nc


def _prep_host(inputs):
    """Host-side weight preformatting (numpy, one-time)."""
    maps = {}
    for st in ("W", "H"):
        wq = np.ascontiguousarray(inputs[f"qkv_w_{st}"], dtype=np.float32)
        bq = np.ascontiguousarray(inputs[f"qkv_b_{st}"], dtype=np.float32)
        dw = np.ascontiguousarray(inputs[f"dw_{st}"], dtype=np.float32)
        gamma = inputs[f"gamma_{st}"].astype(np.float32)
        beta = inputs[f"beta_{st}"].astype(np.float32)
        mean = inputs[f"mean_{st}"].astype(np.float32)
        var = inputs[f"var_{st}"].astype(np.float32)

        maps[f"wvT_{st}"] = np.ascontiguousarray(wq[1 + C :].T)
        maps[f"wkT_{st}"] = np.ascontiguousarray(wq[1 : 1 + C].T)
        maps[f"wqT_{st}"] = np.ascontiguousarray(wq[0:1].T)
        maps[f"bv_{st}"] = np.ascontiguousarray(0.5 * bq[1 + C :])  # tanh trick
        maps[f"bk_{st}"] = np.ascontiguousarray(0.5 * bq[1 : 1 + C])  # ctx2 = ctx/2
        maps[f"dwc_{st}"] = np.ascontiguousarray(dw.reshape(2, 128, 9))
        rstd = 1.0 / np.sqrt(var + BN_EPS)
        maps[f"bns_{st}"] = np.ascontiguousarray(gamma * rstd)
        maps[f"bnsh_{st}"] = np.ascontiguousarray(beta - gamma * mean * rstd)
    return maps


def _get_nc():
    if "nc" not in _CACHE:
        _CACHE["nc"] = _build()
    return _CACHE["nc"]


def kernel(**inputs):
    from concourse import bass_utils

    nc = _get_nc()
    x = np.ascontiguousarray(inputs["x"], dtype=np.float32).reshape(B, C, HW)
    wmap = _prep_host(inputs)
    in_maps = []
    for c in range(NCORES):
        m = dict(wmap)
        m["x"] = x[c * BPC : (c + 1) * BPC]
        in_maps.append(m)
    res = bass_utils.run_bass_kernel_spmd(nc, in_maps, list(range(NCORES)))
    out = np.concatenate([res.results[c]["out"] for c in range(NCORES)], axis=0)
    return out.reshape(B, C, H, W)


# revision 27
# speedup vs baseline: 1.1801x; 1.1801x over previous
"""DualAxisAggAttn Trainium2 kernel (8-core data-parallel over batch).

Reference computation per axis A in {W, H} (x: [B,C,H,W], O = 1+2C):
  qkv = conv1x1(x)                  -> q [B,1,H,W], k,v [B,C,H,W]
  s   = softmax(q, axis=A)
  ctx = sum_A(k * s)                -> [B,C,H,1] or [B,C,1,W]
  out = x + sigmoid(v) * ctx
  y   = SiLU(BN(dwconv3x3(out)))
  x'  = out + y
Axes applied sequentially (W then H).

Kernel strategy per core (2 images):
  - layout: channels on partitions (2 halves of 128), flat h*w on free dim
  - algebraic rewrite: ctx = Wk @ xs + bk with xs[c,a] = sum_A x[c,...]*s[...]
    (k never materialized; q bias dropped: softmax is shift-invariant)
  - PE, fp32r (~1e-4 rounding, 4x faster than fp32): v/q matmuls, ctx matmul,
    9 depthwise taps as diagonal-weight matmuls accumulated in PSUM
  - ACT: tanh((v+bv)/2) (sigmoid via tanh keeps bulk ACT ops in the silu act
    table), SiLU(bn) eviction, softmax exp, psum evictions
  - DVE: xs multiply+reduce, (tanh+1)*ctx2 via scalar_tensor_tensor,
    residual adds via affine_then_add
  - POOL: partition_broadcast of scores, f32->f32r conversion, diag build
  - weights transposed / BN-folded on host (one-time prep)
"""

import numpy as np

B, C, H, W = 16, 256, 80, 80
O = 1 + 2 * C
NCORES = 8
BPC = B // NCORES
HW = H * W
PH, PW = H + 2, W + 2
RCH = 6  # rows per v/dw chunk
NCHUNK = (H + RCH - 1) // RCH  # 14
NPAIR = (NCHUNK + 1) // 2  # 7
XSCH = 10  # rows per xs chunk -> 8 chunks of 800
NXS = H // XSCH
QCH = 5  # q rows per chunk -> 16 chunks of 400
BN_EPS = 1e-5

_CACHE = {}


def _build(n_img=BPC):
    import concourse.bass as bass
    import concourse.bacc as bacc
    import concourse.mybir as mybir
    import concourse.tile as tile
    from concourse import library_config
    from concourse.masks import make_identity

    f32 = mybir.dt.float32
    f32r = mybir.dt.float32r
    Alu = mybir.AluOpType
    Act = mybir.ActivationFunctionType

    nc = bacc.Bacc("TRN2", target_bir_lowering=False, debug=False)

    xd = nc.declare_dram_parameter("x", [n_img, C, HW], f32, isOutput=False)
    prm = {}
    for st in ("W", "H"):
        prm[st] = {
            "wvT": nc.declare_dram_parameter(f"wvT_{st}", [C, C], f32, isOutput=False),
            "wkT": nc.declare_dram_parameter(f"wkT_{st}", [C, C], f32, isOutput=False),
            "wqT": nc.declare_dram_parameter(f"wqT_{st}", [C, 1], f32, isOutput=False),
            "bv": nc.declare_dram_parameter(f"bv_{st}", [C], f32, isOutput=False),
            "bk": nc.declare_dram_parameter(f"bk_{st}", [C], f32, isOutput=False),
            "dwc": nc.declare_dram_parameter(f"dwc_{st}", [2, 128, 9], f32, isOutput=False),
            "bns": nc.declare_dram_parameter(f"bns_{st}", [C], f32, isOutput=False),
            "bnsh": nc.declare_dram_parameter(f"bnsh_{st}", [C], f32, isOutput=False),
        }
    outd = nc.declare_dram_parameter("out", [n_img, C, HW], f32, isOutput=True)

    with tile.TileContext(nc) as tc:
        with (
            tc.tile_pool(name="wgt", bufs=1) as wgt,
            tc.tile_pool(name="wstage", bufs=2) as wstage,
            tc.tile_pool(name="xin", bufs=1) as xin_pool,
            tc.tile_pool(name="xstage", bufs=2) as xstage_pool,
            tc.tile_pool(name="pad", bufs=1) as pad_pool,
            tc.tile_pool(name="sb2", bufs=2) as sb2,
            tc.tile_pool(name="chk", bufs=2) as chk,
            tc.tile_pool(name="sml", bufs=2) as sml,
            tc.tile_pool(name="oq", bufs=2) as oq,
            tc.tile_pool(name="ps_q", bufs=2, space="PSUM") as ps_q,
            tc.tile_pool(name="ps_v", bufs=2, space="PSUM") as ps_v,
            tc.tile_pool(name="ps_dw", bufs=2, space="PSUM") as ps_dw,
            tc.tile_pool(name="ps_x", bufs=1, space="PSUM") as ps_x,
        ):
            nc.gpsimd.load_library(library_config.attn)
            lp = lambda: nc.allow_low_precision(reason="f32r rounding ~1e-4 acceptable")

            ident = wgt.tile([128, 128], f32)
            make_identity(nc, ident)
            zeros = wgt.tile([128, PW], f32)
            nc.vector.memset(zeros[:], 0.0)

            # ---- per-stage constants ----
            SW = {}
            for st in ("W", "H"):
                p = prm[st]
                lv, lk, lq = [], [], []
                for kt in range(2):
                    stg = wstage.tile([128, C], f32, tag="wstg")
                    nc.scalar.dma_start(out=stg[:], in_=p["wvT"][kt * 128 : (kt + 1) * 128, :])
                    tv = wgt.tile([128, C], f32r, tag=f"lv{st}{kt}")
                    nc.vector.tensor_copy(out=tv[:], in_=stg[:])
                    lv.append(tv)
                    stg2 = wstage.tile([128, C], f32, tag="wstg")
                    nc.scalar.dma_start(out=stg2[:], in_=p["wkT"][kt * 128 : (kt + 1) * 128, :])
                    tk = wgt.tile([128, C], f32r, tag=f"lk{st}{kt}")
                    nc.vector.tensor_copy(out=tk[:], in_=stg2[:])
                    lk.append(tk)
                    stg3 = wstage.tile([128, 1], f32, tag="wstgq")
                    nc.scalar.dma_start(out=stg3[:], in_=p["wqT"][kt * 128 : (kt + 1) * 128, :])
                    tq = wgt.tile([128, 1], f32r, tag=f"lq{st}{kt}")
                    nc.vector.tensor_copy(out=tq[:], in_=stg3[:])
                    lq.append(tq)
                bv, bk, bns, bnsh = [], [], [], []
                for mt in range(2):
                    sl = slice(mt * 128, (mt + 1) * 128)
                    for name, lst in (("bv", bv), ("bk", bk), ("bns", bns), ("bnsh", bnsh)):
                        t = wgt.tile([128, 1], f32, tag=f"{name}{st}{mt}")
                        nc.scalar.dma_start(out=t[:], in_=p[name][sl][:, None])
                        lst.append(t)
                dwco = []
                for mt in range(2):
                    cot = wgt.tile([128, 9], f32, tag=f"dwco{st}{mt}")
                    nc.scalar.dma_start(out=cot[:], in_=p["dwc"][mt])
                    dwco.append(cot)
                SW[st] = dict(lv=lv, lk=lk, lq=lq, bv=bv, bk=bk, dwco=dwco, bns=bns, bnsh=bnsh)

            # ---- per image ----
            for img in range(n_img):
                xin = []
                for mt in range(2):
                    t = xin_pool.tile([128, HW], f32r, tag=f"xin{mt}", name=f"xin{mt}")
                    for sp in range(4):
                        stg = xstage_pool.tile([128, 1600], f32, tag="xstg", name="xstg")
                        nc.sync.dma_start(
                            out=stg[:],
                            in_=xd[img, mt * 128 : (mt + 1) * 128, sp * 1600 : (sp + 1) * 1600],
                        )
                        nc.gpsimd.tensor_copy(
                            out=t[:, sp * 1600 : (sp + 1) * 1600], in_=stg[:]
                        )
                    xin.append(t)

                cur = xin
                for st in ("W", "H"):
                    sw = SW[st]
                    # ---------- diag dw weights (f32r, built on POOL) ----------
                    diag = []
                    for mt in range(2):
                        dd = []
                        for t9 in range(9):
                            d = sml.tile(
                                [128, 128], f32r, tag=f"diag{t9}", bufs=1, name=f"diag{t9}"
                            )
                            nc.gpsimd.tensor_scalar_mul(
                                d[:], in0=ident[:], scalar1=sw["dwco"][mt][:, t9 : t9 + 1]
                            )
                            dd.append(d)
                        diag.append(dd)

                    # ---------- padded 'out' tiles (borders zeroed) ----------
                    padded = []
                    for mt in range(2):
                        pt = pad_pool.tile([128, PH, PW], f32r, tag=f"pad{mt}", name=f"pad{mt}")
                        nc.gpsimd.tensor_copy(out=pt[:, 0, :], in_=zeros[:])
                        nc.gpsimd.tensor_copy(out=pt[:, PH - 1, :], in_=zeros[:])
                        nc.gpsimd.tensor_copy(
                            out=pt[:, 1 : PH - 1, 0:1], in_=zeros[:, : PH - 2][:, :, None]
                        )
                        nc.gpsimd.tensor_copy(
                            out=pt[:, 1 : PH - 1, PW - 1 : PW], in_=zeros[:, : PH - 2][:, :, None]
                        )
                        padded.append(pt)

                    # ---------- q = Wq.T @ x ----------
                    q_rows = sml.tile([80, 80], f32, tag="qrows")
                    for chp in range(8):
                        qe = sml.tile([1, 2 * QCH * W], f32, tag="qe")
                        for sub in range(2):
                            ch = chp * 2 + sub
                            n0 = ch * QCH * W
                            pq = ps_q.tile([1, QCH * W], f32, tag="psq")
                            for kt in range(2):
                                nc.tensor.matmul(
                                    pq[:],
                                    sw["lq"][kt][:],
                                    cur[kt][:, n0 : n0 + QCH * W],
                                    start=(kt == 0),
                                    stop=(kt == 1),
                                )
                            nc.scalar.copy(
                                qe[:, sub * QCH * W : (sub + 1) * QCH * W], pq[:]
                            )
                        nc.sync.dma_start(
                            out=q_rows[chp * 2 * QCH : (chp + 1) * 2 * QCH, :], in_=qe[:]
                        )
                    # ---------- softmax ----------
                    if st == "H":
                        ptr = ps_x.tile([80, 80], f32, tag="ptr")
                        nc.tensor.transpose(ptr[:], q_rows[:], ident[:80, :80])
                        q_sm = sml.tile([80, 80], f32, tag="qsm")
                        nc.scalar.copy(q_sm[:], ptr[:])
                    else:
                        q_sm = q_rows
                    nmax = sml.tile([80, 1], f32, tag="nmax")
                    nc.vector.tensor_reduce(
                        nmax[:], q_sm[:], axis=mybir.AxisListType.X, op=Alu.max, negate=True
                    )
                    eq = sml.tile([80, 80], f32, tag="eq")
                    nc.scalar.activation(eq[:], q_sm[:], Act.Exp, bias=nmax[:], scale=1.0)
                    ssum = sml.tile([80, 1], f32, tag="ssum")
                    nc.vector.reduce_sum(ssum[:], eq[:], axis=mybir.AxisListType.X)
                    rs = sml.tile([80, 1], f32, tag="rs")
                    nc.vector.reciprocal(rs[:], ssum[:])
                    s_sm = sml.tile([80, 80], f32, tag="ssm")
                    nc.vector.tensor_scalar_mul(s_sm[:], in0=eq[:], scalar1=rs[:])
                    if st == "H":
                        ptr2 = ps_x.tile([80, 80], f32, tag="ptr")
                        nc.tensor.transpose(ptr2[:], s_sm[:], ident[:80, :80])
                        s_hw = sml.tile([80, 80], f32, tag="shw")
                        nc.scalar.copy(s_hw[:], ptr2[:])
                    else:
                        s_hw = s_sm
                    # ---------- xs = reduce_A(x * s_bcast) ----------
                    xs = [
                        sml.tile([128, H], f32r, tag=f"xs{kt}", name=f"xs{kt}")
                        for kt in range(2)
                    ]
                    if st == "H":
                        xsp = [
                            [
                                sml.tile([128, W], f32, tag=f"xsp{kt}{i}", name=f"xsp{kt}{i}")
                                for i in range(NXS)
                            ]
                            for kt in range(2)
                        ]
                    for ci in range(NXS):
                        c0 = ci * XSCH * W
                        sch = sml.tile([1, XSCH * W], f32, tag="sch")
                        nc.scalar.dma_start(
                            out=sch[:], in_=s_hw[ci * XSCH : (ci + 1) * XSCH, :]
                        )
                        sb = sb2.tile([128, XSCH * W], f32, tag="sbc", bufs=3)
                        nc.gpsimd.partition_broadcast(sb[:], sch[:])
                        for kt in range(2):
                            pr = sb2.tile([128, XSCH * W], f32, tag="prod", bufs=2)
                            nc.vector.tensor_mul(pr[:], cur[kt][:, c0 : c0 + XSCH * W], sb[:])
                            if st == "W":
                                with lp():
                                    nc.vector.reduce_sum(
                                        xs[kt][:, ci * XSCH : (ci + 1) * XSCH],
                                        pr.rearrange("p (h w) -> p h w", h=XSCH),
                                        axis=mybir.AxisListType.X,
                                    )
                            else:
                                nc.vector.reduce_sum(
                                    xsp[kt][ci][:],
                                    pr.rearrange("p (h w) -> p w h", h=XSCH),
                                    axis=mybir.AxisListType.X,
                                )
                    if st == "H":
                        for kt in range(2):
                            for i in range(1, NXS - 1):
                                nc.vector.tensor_add(xsp[kt][0][:], xsp[kt][0][:], xsp[kt][i][:])
                            with lp():
                                nc.vector.tensor_add(xs[kt][:], xsp[kt][0][:], xsp[kt][NXS - 1][:])
                    # ---------- ctx2 = 0.5*(Wk @ xs) + 0.5*bk ----------
                    ctx = []
                    for mt in range(2):
                        pc = ps_x.tile([128, H], f32, tag="psctx")
                        for kt in range(2):
                            nc.tensor.matmul(
                                pc[:],
                                sw["lk"][kt][:, mt * 128 : (mt + 1) * 128],
                                xs[kt][:],
                                start=(kt == 0),
                                stop=(kt == 1),
                            )
                        ct = sml.tile([128, H], f32, tag=f"ctx{mt}")
                        nc.scalar.activation(
                            ct[:], pc[:], Act.Identity, bias=sw["bk"][mt][:], scale=0.5
                        )
                        ctx.append(ct)

                    # ---------- v chunks -> tanh (480) -> t2/out (paired 960) ----------
                    for mt in range(2):
                        for pch in range(NPAIR):
                            r0p = pch * 2 * RCH
                            nrp = min(2 * RCH, H - r0p)
                            th = chk.tile([128, 2 * RCH * W], f32, tag="th")
                            for sub in range(2):
                                ch = pch * 2 + sub
                                r0 = ch * RCH
                                nr = min(RCH, H - r0)
                                if nr <= 0:
                                    continue
                                n0 = r0 * W
                                nn = nr * W
                                pv = ps_v.tile([128, RCH * W], f32, tag="psv")
                                for kt in range(2):
                                    nc.tensor.matmul(
                                        pv[:, :nn],
                                        sw["lv"][kt][:, mt * 128 : (mt + 1) * 128],
                                        cur[kt][:, n0 : n0 + nn],
                                        start=(kt == 0),
                                        stop=(kt == 1),
                                    )
                                nc.scalar.activation(
                                    th[:, sub * RCH * W : sub * RCH * W + nn],
                                    pv[:, :nn], Act.Tanh, bias=sw["bv"][mt][:], scale=0.5,
                                )
                            t2 = chk.tile([128, 2 * RCH, W], f32, tag="t2")
                            if st == "W":
                                cb = ctx[mt][:, r0p : r0p + nrp].broadcast_to([128, nrp, W])
                            else:
                                cb = ctx[mt][:, None, :].broadcast_to([128, nrp, W])
                            # t2 = (th + 1) * ctx2  == sigmoid(v)*ctx
                            nc.vector.scalar_tensor_tensor(
                                t2[:, :nrp, :],
                                in0=th[:, : nrp * W].rearrange("p (h w) -> p h w", w=W),
                                scalar=1.0,
                                in1=cb,
                                op0=Alu.add,
                                op1=Alu.mult,
                            )
                            nc.vector.affine_then_add(
                                padded[mt][:, 1 + r0p : 1 + r0p + nrp, 1 : 1 + W],
                                cur[mt][:, r0p * W : r0p * W + nrp * W].rearrange(
                                    "p (h w) -> p h w", w=W
                                ),
                                t2[:, :nrp, :],
                                scale=1.0,
                                bias=0.0,
                            )

                    # ---------- depthwise conv (PE, 480) + BN+SiLU + residual (960) ----------
                    for mt in range(2):
                        for pch in range(NPAIR):
                            r0p = pch * 2 * RCH
                            nrp = min(2 * RCH, H - r0p)
                            ysil = chk.tile([128, 2 * RCH * W], f32, tag="ysil")
                            for sub in range(2):
                                ch = pch * 2 + sub
                                r0 = ch * RCH
                                nr = min(RCH, H - r0)
                                if nr <= 0:
                                    continue
                                nn = nr * W
                                pdw = ps_dw.tile([128, RCH * W], f32, tag="psdw")
                                t9 = 0
                                for dy in (-1, 0, 1):
                                    for dx in (-1, 0, 1):
                                        rhs = padded[mt][
                                            :, 1 + r0 + dy : 1 + r0 + dy + nr, 1 + dx : 1 + dx + W
                                        ]
                                        nc.tensor.matmul(
                                            pdw[:, :nn],
                                            diag[mt][t9][:],
                                            rhs,
                                            start=(t9 == 0),
                                            stop=(t9 == 8),
                                        )
                                        t9 += 1
                                nc.scalar.activation(
                                    ysil[:, sub * RCH * W : sub * RCH * W + nn],
                                    pdw[:, :nn], Act.Silu,
                                    bias=sw["bnsh"][mt][:], scale=sw["bns"][mt][:],
                                )
                            if st == "W":
                                with lp():
                                    nc.vector.affine_then_add(
                                        cur[mt][:, r0p * W : r0p * W + nrp * W].rearrange(
                                            "p (h w) -> p h w", w=W
                                        ),
                                        padded[mt][:, 1 + r0p : 1 + r0p + nrp, 1 : 1 + W],
                                        ysil[:, : nrp * W].rearrange("p (h w) -> p h w", w=W),
                                        scale=1.0,
                                        bias=0.0,
                                    )
                            else:
                                och = oq.tile([128, 2 * RCH * W], f32, tag="och")
                                nc.vector.affine_then_add(
                                    och[:, : nrp * W].rearrange("p (h w) -> p h w", w=W),
                                    padded[mt][:, 1 + r0p : 1 + r0p + nrp, 1 : 1 + W],
                                    ysil[:, : nrp * W].rearrange("p (h w) -> p h w", w=W),
                                    scale=1.0,
                                    bias=0.0,
                                )
                                nc.scalar.dma_start(
                                    out=outd[
                                        img, mt * 128 : (mt + 1) * 128,
                                        r0p * W : r0p * W + nrp * W,
                                    ],
                                    in_=och[:, : nrp * W],
                                )

    nc.finalize()
    return nc


def _prep_host(inputs):
    """Host-side weight preformatting (numpy, one-time)."""
    maps = {}
    for st in ("W", "H"):
        wq = np.ascontiguousarray(inputs[f"qkv_w_{st}"], dtype=np.float32)
        bq = np.ascontiguousarray(inputs[f"qkv_b_{st}"], dtype=np.float32)
        dw = np.ascontiguousarray(inputs[f"dw_{st}"], dtype=np.float32)
        gamma = inputs[f"gamma_{st}"].astype(np.float32)
        beta = inputs[f"beta_{st}"].astype(np.float32)
        mean = inputs[f"mean_{st}"].astype(np.float32)
        var = inputs[f"var_{st}"].astype(np.float32)

        maps[f"wvT_{st}"] = np.ascontiguousarray(wq[1 + C :].T)
        maps[f"wkT_{st}"] = np.ascontiguousarray(wq[1 : 1 + C].T)
        maps[f"wqT_{st}"] = np.ascontiguousarray(wq[0:1].T)
        maps[f"bv_{st}"] = np.ascontiguousarray(0.5 * bq[1 + C :])  # tanh trick
        maps[f"bk_{st}"] = np.ascontiguousarray(0.5 * bq[1 : 1 + C])  # ctx2 = ctx/2
        maps[f"dwc_{st}"] = np.ascontiguousarray(dw.reshape(2, 128, 9))
        rstd = 1.0 / np.sqrt(var + BN_EPS)
        maps[f"bns_{st}"] = np.ascontiguousarray(gamma * rstd)
        maps[f"bnsh_{st}"] = np.ascontiguousarray(beta - gamma * mean * rstd)
    return maps


def _get_nc():
    if "nc" not in _CACHE:
        _CACHE["nc"] = _build()
    return _CACHE["nc"]


def kernel(**inputs):
    from concourse import bass_utils

    nc = _get_nc()
    x = np.ascontiguousarray(inputs["x"], dtype=np.float32).reshape(B, C, HW)
    wmap = _prep_host(inputs)
    in_maps = []
    for c in range(NCORES):
        m = dict(wmap)
        m["x"] = x[c * BPC : (c + 1) * BPC]
        in_maps.append(m)
    res = bass_utils.run_bass_kernel_spmd(nc, in_maps, list(range(NCORES)))
    out = np.concatenate([res.results[c]["out"] for c in range(NCORES)], axis=0)
    return out.reshape(B, C, H, W)
